# revision 11
# baseline (speedup 1.0000x reference)
"""Trainium2 Bass kernel for the ModelB graph loss.

Strategy (v4): only the tiny ARI branch (n<=50 batches) needs per-batch
sums; every other reduction in the loss is GLOBAL across batches.

  * Dense p-data (all valid [n,n] elements of every batch) is packed
    free-form (zero padding waste) and split evenly across the 8 cores.
    The two dense log-sums use the product-pairing identity
    ln(a)+ln(b)+ln(c)+ln(d) = ln(a*b*c*d): two DVE multiply levels
    (tensor_mul at 2x, tensor_scalar at 4x; scalar_tensor_tensor runs
    at 1x so it is avoided on the hot path) cut the ACT Ln work 4x.
  * adjacency is binary: sum(a*dlt) becomes a sum of logit over
    host-gathered edge positions, and sum((r-a)^2) = sum(r^2)
    - 2*sum_e(r) + n_edges.  Dense `a` never ships.
  * The TensorEngine computes all remaining product-sums into one PSUM
    tile: sum(r^2) as the diagonal of accumulated R_c^T R_c chunks
    (r ships as fp8), sum_e(r) via ones^T R_e column sums, the coord
    huber sums via [d|relu(|d|-1)]^T-squared, and the per-batch ARI
    sums as diagonals of Ps^T X products over a column-disjoint small
    tile.  The PSUM tile is DMAed out raw; the host picks the
    diagonals (pure selection, like summing any stats tensor).
  * Host does layout/gather/dtype packing and the final scalar algebra
    in float64; all O(N^2) float reductions happen on device.
"""

import sys

for _p in ("/opt/trn_rl_repo", "/root/.axon_site/_ro/trn_rl_repo"):
    if _p not in sys.path:
        sys.path.insert(0, _p)

from contextlib import ExitStack

import numpy as np

import concourse.bass as bass  # noqa: F401  (registers engine methods)
import concourse.tile as tile
from concourse import bacc, mybir
from concourse.bass_utils import run_bass_kernel_spmd

N_CORES = 8
B, N, C = 64, 512, 2
EPS = 1e-8

_FT = mybir.dt.float32
_BF = mybir.dt.bfloat16
_F8 = mybir.dt.float8e4
_AF = mybir.ActivationFunctionType
_OP = mybir.AluOpType

import ml_dtypes

_BF_NP = ml_dtypes.bfloat16
_F8_NP = ml_dtypes.float8_e4m3fn

# ---- SPMD-uniform geometry ----
NCH = 2            # xp pipeline chunks (asymmetric ~60/40)
DWS = (592, 396)   # dense cols per quarter-slot, per chunk
EWS = (60, 40)     # edge cols per quarter-slot, per chunk
HWS = tuple(2 * (d + e) for d, e in zip(DWS, EWS))   # (1304, 872)
XPWS = tuple(2 * h for h in HWS)                     # (2608, 1744)
DENSE_QUADS = sum(DWS) * 128   # dense quad positions per core (126464)
EDGE_QUADS = sum(EWS) * 128    # edge quad positions per core  (12800)
RW = 31 * 128      # xr cols (3968)
REW = 400          # edge-r cols
FS = 128           # small-tile cols (column-disjoint batches)
AUXW = REW + 4 * FS  # aux: [re | ps | as | sgn | sel]  (912)
CW = 24            # coord cols per tensor
NS = 4             # stats cols
NREG = 8           # psum regions: r2, re, p2, pa, pd, abs, l1p, coord
PSW = NREG * 128   # psum width (1024)

_build_cache: dict = {}


def _build():
    nc = bacc.Bacc("TRN2", target_bir_lowering=False, debug=False,
                   num_devices=N_CORES)

    xp_in = [nc.dram_tensor(f"xp{i}", [128, XPWS[i]], _BF,
                            kind="ExternalInput").ap() for i in range(NCH)]
    xr_in = nc.dram_tensor("xr", [128, RW], _F8, kind="ExternalInput").ap()
    aux_in = nc.dram_tensor("aux", [128, AUXW], _BF,
                            kind="ExternalInput").ap()
    crd_in = nc.dram_tensor("crd", [128, 2 * CW], _BF,
                            kind="ExternalInput").ap()
    st_out = nc.dram_tensor("st", [128, NS], _FT,
                            kind="ExternalOutput").ap()
    ps_out = nc.dram_tensor("ps", [128, PSW], _FT,
                            kind="ExternalOutput").ap()

    with tile.TileContext(nc) as tc, ExitStack() as ctx:
        pin = ctx.enter_context(tc.tile_pool(name="pin", bufs=NCH))
        pmid = ctx.enter_context(tc.tile_pool(name="pmid", bufs=NCH))
        pscr = ctx.enter_context(tc.tile_pool(name="pscr", bufs=NCH))
        psml = ctx.enter_context(tc.tile_pool(name="psml", bufs=1))
        pstat = ctx.enter_context(tc.tile_pool(name="pstat", bufs=1))
        ppsum = ctx.enter_context(tc.tile_pool(name="ppsum", bufs=1,
                                               space="PSUM"))

        st = pstat.tile([128, NS], _FT, tag="st")

        def stc(i):
            return st[:, i:i + 1]

        # ---- input DMAs ----
        txp = []
        for i in range(NCH):
            t = pin.tile([128, XPWS[i]], _BF, tag=f"txp{i}")
            nc.sync.dma_start(t[:], xp_in[i][:])
            txp.append(t)
        txr = psml.tile([128, RW], _F8, tag="txr")
        nc.gpsimd.dma_start(txr[:], xr_in[:])
        taux = psml.tile([128, AUXW], _BF, tag="taux")
        nc.gpsimd.dma_start(taux[:], aux_in[:])
        tcrd = psml.tile([128, 2 * CW], _BF, tag="tcrd")
        nc.gpsimd.dma_start(tcrd[:], crd_in[:])

        ones = psml.tile([128, 128], _BF, tag="ones")
        nc.gpsimd.memset(ones[:], 1.0)
        zb = psml.tile([128, CW], _BF, tag="zb")
        nc.gpsimd.memset(zb[:], 0.0)

        psum = ppsum.tile([128, PSW], _FT, tag="psum")

        def reg(i):
            return psum[:, 128 * i:128 * (i + 1)]

        # ---- PE: sum(r^2) via accumulated R^T R diagonal (region 0) ----
        for c in range(31):
            nc.tensor.matmul(reg(0), txr[:, 128 * c:128 * (c + 1)],
                             txr[:, 128 * c:128 * (c + 1)],
                             start=(c == 0), stop=(c == 30))

        # ---- dense+edge paired log chains, per chunk ----
        def chunk_ops(i):
            x = txp[i]
            hw, dw, ew = HWS[i], DWS[i], EWS[i]
            lo = x[:, 0:hw]
            hi = x[:, hw:2 * hw]
            p2 = pmid.tile([128, hw], _BF, tag=f"p2{i}")
            nc.vector.tensor_mul(p2[:], lo, hi)
            mlo = pmid.tile([128, hw], _BF, tag=f"mlo{i}")
            nc.vector.tensor_scalar_sub(mlo[:], lo, 1.0)
            mhi = pmid.tile([128, hw], _BF, tag=f"mhi{i}")
            nc.vector.tensor_scalar_sub(mhi[:], hi, 1.0)
            q2 = pmid.tile([128, hw], _BF, tag=f"q2{i}")
            nc.vector.tensor_mul(q2[:], mlo[:], mhi[:])
            p4 = pmid.tile([128, hw // 2], _BF, tag=f"p4{i}")
            nc.vector.tensor_mul(p4[:], p2[:, 0:hw // 2], p2[:, hw // 2:hw])
            q4 = pmid.tile([128, hw // 2], _BF, tag=f"q4{i}")
            nc.vector.tensor_mul(q4[:], q2[:, 0:hw // 2], q2[:, hw // 2:hw])

            lp4 = pscr.tile([128, hw // 2], _BF, tag=f"lp4{i}")
            nc.scalar.activation(lp4[:], p4[:], _AF.Ln)
            lq4 = pscr.tile([128, hw // 2], _BF, tag=f"lq4{i}")
            nc.scalar.activation(lq4[:], q4[:], _AF.Ln)

            dcmb = pscr.tile([128, dw], _BF, tag=f"dcmb{i}")
            # (lp4/19 + lq4); host multiplies by 0.95
            nc.vector.scalar_tensor_tensor(
                dcmb[:], lp4[:, 0:dw], 1.0 / 19.0, lq4[:, 0:dw],
                _OP.mult, _OP.add, accum_out=stc(0 + i))
            ecmb = pscr.tile([128, ew], _BF, tag=f"ecmb{i}")
            nc.vector.scalar_tensor_tensor(
                ecmb[:], lp4[:, dw:dw + ew], 1.0, lq4[:, dw:dw + ew],
                _OP.mult, _OP.subtract, accum_out=stc(2 + i))

        chunk_ops(0)

        # ---- PE: sum_e(r) via ones^T R_e column sums (region 1) ----
        for c in range(4):
            w = min(128, REW - 128 * c)
            nc.tensor.matmul(reg(1)[:, 0:w], ones[:],
                             taux[:, 128 * c:128 * c + w],
                             start=(c == 0), stop=(c == 3))

        # ---- coords: d = pc - pt; hb = relu(|d|-1); PE squares (reg 7) ----
        dhb = psml.tile([128, 128], _BF, tag="dhb")
        nc.gpsimd.memset(dhb[:], 0.0)
        nc.gpsimd.tensor_sub(dhb[:, 0:CW], tcrd[:, 0:CW], tcrd[:, CW:2 * CW])
        negd = psml.tile([128, CW], _BF, tag="negd")
        nc.vector.tensor_scalar_mul(negd[:], dhb[:, 0:CW], -1.0)
        ad = psml.tile([128, CW], _BF, tag="ad")
        nc.vector.tensor_tensor(ad[:], dhb[:, 0:CW], negd[:], _OP.max)
        adm1 = psml.tile([128, CW], _BF, tag="adm1")
        nc.vector.tensor_scalar_sub(adm1[:], ad[:], 1.0)
        nc.vector.tensor_tensor(dhb[:, CW:2 * CW], adm1[:], zb[:], _OP.max)
        nc.tensor.matmul(reg(7), dhb[:], dhb[:], start=True, stop=True)

        # ---- small (ARI) tile: column-disjoint; PE diagonals (reg 2-6) ----
        ps_ = taux[:, REW:REW + FS]
        as_ = taux[:, REW + FS:REW + 2 * FS]
        sgn_ = taux[:, REW + 2 * FS:REW + 3 * FS]
        sel_ = taux[:, REW + 3 * FS:REW + 4 * FS]
        lps = psml.tile([128, FS], _BF, tag="lps")
        nc.scalar.activation(lps[:], ps_, _AF.Ln)
        lqs = psml.tile([128, FS], _BF, tag="lqs")
        nc.scalar.activation(lqs[:], ps_, _AF.Ln, bias=1.0, scale=-1.0)
        dlt = psml.tile([128, FS], _BF, tag="dlt")
        nc.gpsimd.tensor_sub(dlt[:], lps[:], lqs[:])

        nc.tensor.matmul(reg(2), ps_, ps_, start=True, stop=True)
        nc.tensor.matmul(reg(3), ps_, as_, start=True, stop=True)
        nc.tensor.matmul(reg(4), ps_, dlt[:], start=True, stop=True)
        nc.tensor.matmul(reg(5), ps_, sgn_, start=True, stop=True)
        nc.tensor.matmul(reg(6), sel_, lqs[:], start=True, stop=True)

        # psum copy+dump hide under the chunk-1 DVE/ACT chain
        sps = pstat.tile([128, PSW], _FT, tag="sps")
        nc.scalar.activation(sps[:], psum[:], _AF.Copy)
        nc.sync.dma_start(ps_out[:], sps[:])

        chunk_ops(1)

        nc.sync.dma_start(st_out[:], st[:])

    nc.compile()
    return nc


def _pack_quads(stream, quads_per_chunk, pad):
    """stream (f32) -> per-chunk (loA, hiA, loB, hiB) f32 planes.

    Quad k holds stream[4k:4k+4]; all planes are plain values.
    """
    cap = 4 * sum(quads_per_chunk)
    s = np.full(cap, pad, np.float32)
    s[:len(stream)] = stream
    q = s.reshape(-1, 4)
    out = []
    base = 0
    for nq in quads_per_chunk:
        qc = q[base:base + nq]
        base += nq
        w = nq // 128
        out.append(tuple(qc[:, k].reshape(128, w) for k in range(4)))
    return out


def _even_split(arr, k):
    bounds = np.linspace(0, len(arr), k + 1).astype(np.int64)
    return [arr[bounds[i]:bounds[i + 1]] for i in range(k)]


def _huber(x):
    ax = np.abs(x)
    return np.where(ax <= 1.0, 0.5 * x * x, ax - 0.5)


def kernel(predicted_coords, adjacency_matrix, node_counts, raw_similarity,
           temperature, residual_weight, points, adjacency, node_masks,
           _want_results=None):
    masks = np.asarray(node_masks).astype(bool)
    n_list = masks.sum(axis=1).astype(np.int64)

    if "nc" not in _build_cache:
        _build_cache["nc"] = _build()
    nc = _build_cache["nc"]

    p_full = np.asarray(adjacency_matrix, dtype=np.float32)
    a_full = np.asarray(adjacency, dtype=np.float32)
    r_full = np.asarray(raw_similarity, dtype=np.float32)
    pc_full = np.ascontiguousarray(predicted_coords, dtype=np.float32)
    pt_full = np.ascontiguousarray(points, dtype=np.float32)

    # ---- host-side selection / packing (no float math on data) ----
    valid = []
    for b in range(B):
        n = int(n_list[b])
        valid.append(None if masks[b, :n].all() else np.flatnonzero(masks[b]))

    p_blocks, r_blocks, pe_blocks, re_blocks = [], [], [], []
    ec_list = []
    pcv, ptv = [], []
    for b in range(B):
        n = int(n_list[b])
        if valid[b] is None:
            pb = p_full[b, :n, :n]
            ab = a_full[b, :n, :n]
            rb = r_full[b, :n, :n]
            pcb = pc_full[b, :n]
            ptb = pt_full[b, :n]
        else:
            ix = np.ix_(valid[b], valid[b])
            pb = p_full[b][ix]
            ab = a_full[b][ix]
            rb = r_full[b][ix]
            pcb = pc_full[b][valid[b]]
            ptb = pt_full[b][valid[b]]
        e = ab > 0.5
        p_blocks.append(pb.ravel())
        r_blocks.append(rb.ravel())
        pe_blocks.append(pb[e])
        re_blocks.append(rb[e])
        ec_list.append(int(e.sum()))
        pcv.append(pcb.ravel())
        ptv.append(ptb.ravel())

    dense_p = np.concatenate(p_blocks)
    dense_r = np.concatenate(r_blocks)
    edge_p = np.concatenate(pe_blocks)
    edge_r = np.concatenate(re_blocks)
    pc_s = np.concatenate(pcv)
    pt_s = np.concatenate(ptv)
    e_tot = float(sum(ec_list))

    dense_p_sp = _even_split(dense_p, N_CORES)
    dense_r_sp = _even_split(dense_r, N_CORES)
    edge_p_sp = _even_split(edge_p, N_CORES)
    edge_r_sp = _even_split(edge_r, N_CORES)
    pc_sp = _even_split(pc_s, N_CORES)
    pt_sp = _even_split(pt_s, N_CORES)

    # small (ARI) batches: min-fill greedy into 8 cores, cap 128 cols
    small = [b for b in range(B) if n_list[b] <= 50]
    order = sorted(small, key=lambda b: -n_list[b])
    bins = [[] for _ in range(N_CORES)]
    fill = [0] * N_CORES
    for b in order:
        k = min(range(N_CORES), key=lambda i: fill[i])
        assert fill[k] + n_list[b] <= FS, "small batches don't fit"
        bins[k].append(b)
        fill[k] += int(n_list[b])

    in_maps = []
    small_layout = []
    for c in range(N_CORES):
        im = {}
        dq = _pack_quads(dense_p_sp[c], [d * 128 for d in DWS], 0.5)
        eq = _pack_quads(edge_p_sp[c], [e * 128 for e in EWS], 0.5)
        for i in range(NCH):
            loA, hiA, loB, hiB = dq[i]
            eloA, ehiA, eloB, ehiB = eq[i]
            xp = np.concatenate(
                [loA, eloA, loB, eloB, hiA, ehiA, hiB, ehiB], axis=1)
            im[f"xp{i}"] = xp.astype(_BF_NP)

        rv = np.zeros(128 * RW, np.float32)
        rv[:len(dense_r_sp[c])] = dense_r_sp[c]
        im["xr"] = rv.reshape(128, RW).astype(_F8_NP)

        aux = np.zeros((128, AUXW), np.float32)
        aux[:, REW:REW + FS] = 0.5
        rev = edge_r_sp[c]
        rem = np.zeros(128 * REW, np.float32)
        rem[:len(rev)] = rev
        aux[:, 0:REW] = rem.reshape(128, REW)
        lay = []
        off = 0
        for b in bins[c]:
            n = int(n_list[b])
            if valid[b] is None:
                pb = p_full[b, :n, :n]
                ab = a_full[b, :n, :n]
            else:
                ix = np.ix_(valid[b], valid[b])
                pb = p_full[b][ix]
                ab = a_full[b][ix]
            pb_bf = pb.astype(_BF_NP).astype(np.float32)
            sg = np.where(pb_bf >= 0.5, 1.0, -1.0).astype(np.float32)
            aux[0:n, REW + off:REW + off + n] = pb
            aux[0:n, REW + FS + off:REW + FS + off + n] = ab
            aux[0:n, REW + 2 * FS + off:REW + 2 * FS + off + n] = sg
            aux[0:n, REW + 3 * FS + off:REW + 3 * FS + off + n] = 1.0
            lay.append((b, off, n, float(sg.sum())))
            off += n
        small_layout.append(lay)
        im["aux"] = aux.astype(_BF_NP)

        crd = np.zeros((128, 2 * CW), np.float32)
        v = np.zeros(128 * CW, np.float32)
        v[:len(pc_sp[c])] = pc_sp[c]
        crd[:, 0:CW] = v.reshape(128, CW)
        v = np.zeros(128 * CW, np.float32)
        v[:len(pt_sp[c])] = pt_sp[c]
        crd[:, CW:2 * CW] = v.reshape(128, CW)
        im["crd"] = crd.astype(_BF_NP)
        in_maps.append(im)

    res = run_bass_kernel_spmd(nc, in_maps, core_ids=list(range(N_CORES)))
    if _want_results is not None:
        _want_results.append(res)

    # ---- host finalization (float64) ----
    st = [res.results[c]["st"].astype(np.float64) for c in range(N_CORES)]
    dg = np.arange(128)
    pdiag = [res.results[c]["ps"].astype(np.float64) for c in range(N_CORES)]

    def diag(c, r):
        m = pdiag[c][:, 128 * r:128 * (r + 1)]
        return m[dg, dg]

    n_arr = n_list.astype(np.float64)
    cnt_coord = max(float(n_arr.sum()) * C, 1.0)
    cnt2 = max(float((n_arr ** 2).sum()), 1.0)
    LNH = float(np.log(0.5))

    s_dense = sum(float(s[:, 0].sum() + s[:, 1].sum()) for s in st)
    s_edge = sum(float(s[:, 2].sum() + s[:, 3].sum()) for s in st)
    npad_dense = 4.0 * DENSE_QUADS * N_CORES - len(dense_p)
    s_dense -= npad_dense * (20.0 / 19.0) * LNH
    edge_sum = 0.95 * s_dense + 0.9 * s_edge
    edge_loss = -edge_sum / cnt2

    s_r2 = sum(float(diag(c, 0).sum()) for c in range(N_CORES))
    s_re = sum(float(diag(c, 1).sum()) for c in range(N_CORES))
    similarity_loss = (s_r2 - 2.0 * s_re + e_tot) / cnt2

    s_d2 = sum(float(diag(c, 7)[0:CW].sum()) for c in range(N_CORES))
    s_h2 = sum(float(diag(c, 7)[CW:2 * CW].sum()) for c in range(N_CORES))
    coord_mse = s_d2 / cnt_coord
    coord_smooth = 0.5 * (s_d2 - s_h2) / cnt_coord
    coord_loss = 0.7 * coord_mse + 0.3 * coord_smooth

    ari_loss = 0.0
    conf_pen = 0.0
    for c in range(N_CORES):
        d2 = diag(c, 2)
        d3 = diag(c, 3)
        d4 = diag(c, 4)
        d5 = diag(c, 5)
        d6 = diag(c, 6)
        for b, off, n, sgsum in small_layout[c]:
            nf = float(n)
            sl = slice(off, off + n)
            s_p2 = float(d2[sl].sum()) - nf * (FS - nf) * 0.25
            s_pa = float(d3[sl].sum())
            s_pd = float(d4[sl].sum())
            s_abs = float(d5[sl].sum()) - 0.5 * sgsum
            s_l1p = float(d6[sl].sum())
            if not (5.0 < nf <= 50.0):
                continue
            na = np.sqrt(max(s_p2, 0.0))
            nt = np.sqrt(max(float(ec_list[b]), 0.0))
            cos = s_pa / (max(na, EPS) * max(nt, EPS))
            n2 = max(nf * nf, 1.0)
            ent = -(s_l1p + s_pd) / n2
            contrast = s_abs / n2
            ari_loss += -cos - 0.2 * contrast
            conf_pen += ent

    dc = np.asarray(node_counts, np.float64) - n_arr
    count_loss = float(_huber(dc).mean())
    temp_reg = abs(float(temperature) - 1.0)
    res_reg = abs(float(residual_weight) - 0.5)

    total = (1.0 * coord_loss + 2.0 * edge_loss + 0.1 * count_loss
             + 0.3 * similarity_loss + 0.01 * (temp_reg + res_reg)
             + 1.0 * (ari_loss + 0.1 * conf_pen))
    return np.asarray(total, dtype=np.float32)


# revision 14
# speedup vs baseline: 1.0688x; 1.0688x over previous
"""Trainium2 Bass kernel for the ModelB graph loss.

Strategy (v4): only the tiny ARI branch (n<=50 batches) needs per-batch
sums; every other reduction in the loss is GLOBAL across batches.

  * Dense p-data (all valid [n,n] elements of every batch) is packed
    free-form (zero padding waste) and split evenly across the 8 cores.
    The two dense log-sums use the product-pairing identity
    ln(a)+ln(b)+ln(c)+ln(d) = ln(a*b*c*d): two DVE multiply levels
    (tensor_mul at 2x, tensor_scalar at 4x; scalar_tensor_tensor runs
    at 1x so it is avoided on the hot path) cut the ACT Ln work 4x.
  * adjacency is binary: sum(a*dlt) becomes a sum of logit over
    host-gathered edge positions, and sum((r-a)^2) = sum(r^2)
    - 2*sum_e(r) + n_edges.  Dense `a` never ships.
  * The TensorEngine computes all remaining product-sums into one PSUM
    tile: sum(r^2) as the diagonal of accumulated R_c^T R_c chunks
    (r ships as fp8), sum_e(r) via ones^T R_e column sums, the coord
    huber sums via [d|relu(|d|-1)]^T-squared, and the per-batch ARI
    sums as diagonals of Ps^T X products over a column-disjoint small
    tile.  The PSUM tile is DMAed out raw; the host picks the
    diagonals (pure selection, like summing any stats tensor).
  * Host does layout/gather/dtype packing and the final scalar algebra
    in float64; all O(N^2) float reductions happen on device.
"""

import sys

for _p in ("/opt/trn_rl_repo", "/root/.axon_site/_ro/trn_rl_repo"):
    if _p not in sys.path:
        sys.path.insert(0, _p)

from contextlib import ExitStack

import numpy as np

import concourse.bass as bass  # noqa: F401  (registers engine methods)
import concourse.tile as tile
from concourse import bacc, mybir
from concourse.bass_utils import run_bass_kernel_spmd

N_CORES = 8
B, N, C = 64, 512, 2
EPS = 1e-8

_FT = mybir.dt.float32
_BF = mybir.dt.bfloat16
_F8 = mybir.dt.float8e4
_AF = mybir.ActivationFunctionType
_OP = mybir.AluOpType

import ml_dtypes

_BF_NP = ml_dtypes.bfloat16
_F8_NP = ml_dtypes.float8_e4m3fn

# ---- SPMD-uniform geometry ----
NCH = 3            # xp pipeline chunks (asymmetric ~16/44/40)
DWS = (158, 434, 396)  # dense cols per quarter-slot, per chunk
EWS = (16, 44, 40)     # edge cols per quarter-slot, per chunk
HWS = tuple(2 * (d + e) for d, e in zip(DWS, EWS))   # (1304, 872)
XPWS = tuple(2 * h for h in HWS)                     # (2608, 1744)
DENSE_QUADS = sum(DWS) * 128   # dense quad positions per core (126464)
EDGE_QUADS = sum(EWS) * 128    # edge quad positions per core  (12800)
RW = 31 * 128      # xr cols (3968)
REW = 400          # edge-r cols
FS = 128           # small-tile cols (column-disjoint batches)
AUXW = REW + 4 * FS  # aux: [re | ps | as | sgn | sel]  (912)
CW = 24            # coord cols per tensor
NS = 2 * NCH       # stats cols: dense[0:NCH], edge[NCH:2NCH]
NREG = 8           # psum regions: r2, re, p2, pa, pd, abs, l1p, coord
PSW = NREG * 128   # psum width (1024)

_build_cache: dict = {}


def _build():
    nc = bacc.Bacc("TRN2", target_bir_lowering=False, debug=False,
                   num_devices=N_CORES)

    xp_in = [nc.dram_tensor(f"xp{i}", [128, XPWS[i]], _BF,
                            kind="ExternalInput").ap() for i in range(NCH)]
    xr_in = nc.dram_tensor("xr", [128, RW], _F8, kind="ExternalInput").ap()
    aux_in = nc.dram_tensor("aux", [128, AUXW], _BF,
                            kind="ExternalInput").ap()
    crd_in = nc.dram_tensor("crd", [128, 2 * CW], _BF,
                            kind="ExternalInput").ap()
    st_out = nc.dram_tensor("st", [128, NS], _FT,
                            kind="ExternalOutput").ap()
    ps_out = nc.dram_tensor("ps", [128, PSW], _FT,
                            kind="ExternalOutput").ap()

    with tile.TileContext(nc) as tc, ExitStack() as ctx:
        pin = ctx.enter_context(tc.tile_pool(name="pin", bufs=NCH))
        pmid = ctx.enter_context(tc.tile_pool(name="pmid", bufs=NCH))
        pscr = ctx.enter_context(tc.tile_pool(name="pscr", bufs=NCH))
        psml = ctx.enter_context(tc.tile_pool(name="psml", bufs=1))
        pstat = ctx.enter_context(tc.tile_pool(name="pstat", bufs=1))
        ppsum = ctx.enter_context(tc.tile_pool(name="ppsum", bufs=1,
                                               space="PSUM"))

        st = pstat.tile([128, NS], _FT, tag="st")

        def stc(i):
            return st[:, i:i + 1]

        # ---- input DMAs ----
        txp = []
        for i in range(NCH):
            t = pin.tile([128, XPWS[i]], _BF, tag=f"txp{i}")
            nc.sync.dma_start(t[:], xp_in[i][:])
            txp.append(t)
        txr = psml.tile([128, RW], _F8, tag="txr")
        nc.gpsimd.dma_start(txr[:], xr_in[:])
        taux = psml.tile([128, AUXW], _BF, tag="taux")
        nc.gpsimd.dma_start(taux[:], aux_in[:])
        tcrd = psml.tile([128, 2 * CW], _BF, tag="tcrd")
        nc.gpsimd.dma_start(tcrd[:], crd_in[:])

        ones = psml.tile([128, 128], _BF, tag="ones")
        nc.gpsimd.memset(ones[:], 1.0)
        bm1 = psml.tile([128, 1], _FT, tag="bm1")
        nc.gpsimd.memset(bm1[:], -1.0)

        psum = ppsum.tile([128, PSW], _FT, tag="psum")

        def reg(i):
            return psum[:, 128 * i:128 * (i + 1)]

        # ---- PE: sum(r^2) via accumulated R^T R diagonal (region 0) ----
        for c in range(31):
            nc.tensor.matmul(reg(0), txr[:, 128 * c:128 * (c + 1)],
                             txr[:, 128 * c:128 * (c + 1)],
                             start=(c == 0), stop=(c == 30))

        # ---- dense+edge paired log chains, per chunk ----
        def chunk_ops(i):
            x = txp[i]
            hw, dw, ew = HWS[i], DWS[i], EWS[i]
            lo = x[:, 0:hw]
            hi = x[:, hw:2 * hw]
            p2 = pmid.tile([128, hw], _BF, tag=f"p2{i}")
            nc.vector.tensor_mul(p2[:], lo, hi)
            mlo = pmid.tile([128, hw], _BF, tag=f"mlo{i}")
            nc.vector.tensor_scalar_sub(mlo[:], lo, 1.0)
            mhi = pmid.tile([128, hw], _BF, tag=f"mhi{i}")
            nc.vector.tensor_scalar_sub(mhi[:], hi, 1.0)
            q2 = pmid.tile([128, hw], _BF, tag=f"q2{i}")
            nc.vector.tensor_mul(q2[:], mlo[:], mhi[:])
            p4 = pmid.tile([128, hw // 2], _BF, tag=f"p4{i}")
            nc.vector.tensor_mul(p4[:], p2[:, 0:hw // 2], p2[:, hw // 2:hw])
            q4 = pmid.tile([128, hw // 2], _BF, tag=f"q4{i}")
            nc.vector.tensor_mul(q4[:], q2[:, 0:hw // 2], q2[:, hw // 2:hw])

            lp4 = pscr.tile([128, hw // 2], _BF, tag=f"lp4{i}")
            nc.scalar.activation(lp4[:], p4[:], _AF.Ln)
            lq4 = pscr.tile([128, hw // 2], _BF, tag=f"lq4{i}")
            nc.scalar.activation(lq4[:], q4[:], _AF.Ln)

            dcmb = pscr.tile([128, dw], _BF, tag=f"dcmb{i}")
            # (lp4/19 + lq4); host multiplies by 0.95
            nc.vector.scalar_tensor_tensor(
                dcmb[:], lp4[:, 0:dw], 1.0 / 19.0, lq4[:, 0:dw],
                _OP.mult, _OP.add, accum_out=stc(i))
            ecmb = pscr.tile([128, ew], _BF, tag=f"ecmb{i}")
            nc.vector.scalar_tensor_tensor(
                ecmb[:], lp4[:, dw:dw + ew], 1.0, lq4[:, dw:dw + ew],
                _OP.mult, _OP.subtract, accum_out=stc(NCH + i))

        chunk_ops(0)

        # ---- PE: sum_e(r) via ones^T R_e column sums (region 1) ----
        for c in range(4):
            w = min(128, REW - 128 * c)
            nc.tensor.matmul(reg(1)[:, 0:w], ones[:],
                             taux[:, 128 * c:128 * c + w],
                             start=(c == 0), stop=(c == 3))

        # ---- coords: d = pc - pt; hb = relu(|d|-1); PE squares (reg 7) ----
        dhb = psml.tile([128, 128], _BF, tag="dhb")
        nc.gpsimd.memset(dhb[:], 0.0)
        nc.gpsimd.tensor_sub(dhb[:, 0:CW], tcrd[:, 0:CW], tcrd[:, CW:2 * CW])
        ad = psml.tile([128, CW], _BF, tag="ad")
        nc.scalar.activation(ad[:], dhb[:, 0:CW], _AF.Abs)
        nc.scalar.activation(dhb[:, CW:2 * CW], ad[:], _AF.Relu,
                             bias=bm1[:])
        nc.tensor.matmul(reg(7), dhb[:], dhb[:], start=True, stop=True)

        # ---- small (ARI) tile: column-disjoint; PE diagonals (reg 2-6) ----
        ps_ = taux[:, REW:REW + FS]
        as_ = taux[:, REW + FS:REW + 2 * FS]
        sgn_ = taux[:, REW + 2 * FS:REW + 3 * FS]
        sel_ = taux[:, REW + 3 * FS:REW + 4 * FS]
        lps = psml.tile([128, FS], _BF, tag="lps")
        nc.scalar.activation(lps[:], ps_, _AF.Ln)
        lqs = psml.tile([128, FS], _BF, tag="lqs")
        nc.scalar.activation(lqs[:], ps_, _AF.Ln, bias=1.0, scale=-1.0)
        dlt = psml.tile([128, FS], _BF, tag="dlt")
        nc.gpsimd.tensor_sub(dlt[:], lps[:], lqs[:])

        nc.tensor.matmul(reg(2), ps_, ps_, start=True, stop=True)
        nc.tensor.matmul(reg(3), ps_, as_, start=True, stop=True)
        nc.tensor.matmul(reg(4), ps_, dlt[:], start=True, stop=True)
        nc.tensor.matmul(reg(5), ps_, sgn_, start=True, stop=True)
        nc.tensor.matmul(reg(6), sel_, lqs[:], start=True, stop=True)

        chunk_ops(1)

        # psum copy+dump slot into the ACT bubble before chunk 2's LNs
        sps = pstat.tile([128, PSW], _FT, tag="sps")
        nc.scalar.activation(sps[:], psum[:], _AF.Copy)
        nc.sync.dma_start(ps_out[:], sps[:])

        chunk_ops(2)

        nc.sync.dma_start(st_out[:], st[:])

    nc.compile()
    return nc


def _pack_quads(stream, quads_per_chunk, pad):
    """stream (f32) -> per-chunk (loA, hiA, loB, hiB) f32 planes.

    Quad k holds stream[4k:4k+4]; all planes are plain values.
    """
    cap = 4 * sum(quads_per_chunk)
    s = np.full(cap, pad, np.float32)
    s[:len(stream)] = stream
    q = s.reshape(-1, 4)
    out = []
    base = 0
    for nq in quads_per_chunk:
        qc = q[base:base + nq]
        base += nq
        w = nq // 128
        out.append(tuple(qc[:, k].reshape(128, w) for k in range(4)))
    return out


def _even_split(arr, k):
    bounds = np.linspace(0, len(arr), k + 1).astype(np.int64)
    return [arr[bounds[i]:bounds[i + 1]] for i in range(k)]


def _huber(x):
    ax = np.abs(x)
    return np.where(ax <= 1.0, 0.5 * x * x, ax - 0.5)


def kernel(predicted_coords, adjacency_matrix, node_counts, raw_similarity,
           temperature, residual_weight, points, adjacency, node_masks,
           _want_results=None):
    masks = np.asarray(node_masks).astype(bool)
    n_list = masks.sum(axis=1).astype(np.int64)

    if "nc" not in _build_cache:
        _build_cache["nc"] = _build()
    nc = _build_cache["nc"]

    p_full = np.asarray(adjacency_matrix, dtype=np.float32)
    a_full = np.asarray(adjacency, dtype=np.float32)
    r_full = np.asarray(raw_similarity, dtype=np.float32)
    pc_full = np.ascontiguousarray(predicted_coords, dtype=np.float32)
    pt_full = np.ascontiguousarray(points, dtype=np.float32)

    # ---- host-side selection / packing (no float math on data) ----
    valid = []
    for b in range(B):
        n = int(n_list[b])
        valid.append(None if masks[b, :n].all() else np.flatnonzero(masks[b]))

    p_blocks, r_blocks, pe_blocks, re_blocks = [], [], [], []
    ec_list = []
    pcv, ptv = [], []
    for b in range(B):
        n = int(n_list[b])
        if valid[b] is None:
            pb = p_full[b, :n, :n]
            ab = a_full[b, :n, :n]
            rb = r_full[b, :n, :n]
            pcb = pc_full[b, :n]
            ptb = pt_full[b, :n]
        else:
            ix = np.ix_(valid[b], valid[b])
            pb = p_full[b][ix]
            ab = a_full[b][ix]
            rb = r_full[b][ix]
            pcb = pc_full[b][valid[b]]
            ptb = pt_full[b][valid[b]]
        e = ab > 0.5
        p_blocks.append(pb.ravel())
        r_blocks.append(rb.ravel())
        pe_blocks.append(pb[e])
        re_blocks.append(rb[e])
        ec_list.append(int(e.sum()))
        pcv.append(pcb.ravel())
        ptv.append(ptb.ravel())

    dense_p = np.concatenate(p_blocks)
    dense_r = np.concatenate(r_blocks)
    edge_p = np.concatenate(pe_blocks)
    edge_r = np.concatenate(re_blocks)
    pc_s = np.concatenate(pcv)
    pt_s = np.concatenate(ptv)
    e_tot = float(sum(ec_list))

    dense_p_sp = _even_split(dense_p, N_CORES)
    dense_r_sp = _even_split(dense_r, N_CORES)
    edge_p_sp = _even_split(edge_p, N_CORES)
    edge_r_sp = _even_split(edge_r, N_CORES)
    pc_sp = _even_split(pc_s, N_CORES)
    pt_sp = _even_split(pt_s, N_CORES)

    # small (ARI) batches: min-fill greedy into 8 cores, cap 128 cols
    small = [b for b in range(B) if n_list[b] <= 50]
    order = sorted(small, key=lambda b: -n_list[b])
    bins = [[] for _ in range(N_CORES)]
    fill = [0] * N_CORES
    for b in order:
        k = min(range(N_CORES), key=lambda i: fill[i])
        assert fill[k] + n_list[b] <= FS, "small batches don't fit"
        bins[k].append(b)
        fill[k] += int(n_list[b])

    in_maps = []
    small_layout = []
    for c in range(N_CORES):
        im = {}
        dq = _pack_quads(dense_p_sp[c], [d * 128 for d in DWS], 0.5)
        eq = _pack_quads(edge_p_sp[c], [e * 128 for e in EWS], 0.5)
        for i in range(NCH):
            loA, hiA, loB, hiB = dq[i]
            eloA, ehiA, eloB, ehiB = eq[i]
            xp = np.concatenate(
                [loA, eloA, loB, eloB, hiA, ehiA, hiB, ehiB], axis=1)
            im[f"xp{i}"] = xp.astype(_BF_NP)

        rv = np.zeros(128 * RW, np.float32)
        rv[:len(dense_r_sp[c])] = dense_r_sp[c]
        im["xr"] = rv.reshape(128, RW).astype(_F8_NP)

        aux = np.zeros((128, AUXW), np.float32)
        aux[:, REW:REW + FS] = 0.5
        rev = edge_r_sp[c]
        rem = np.zeros(128 * REW, np.float32)
        rem[:len(rev)] = rev
        aux[:, 0:REW] = rem.reshape(128, REW)
        lay = []
        off = 0
        for b in bins[c]:
            n = int(n_list[b])
            if valid[b] is None:
                pb = p_full[b, :n, :n]
                ab = a_full[b, :n, :n]
            else:
                ix = np.ix_(valid[b], valid[b])
                pb = p_full[b][ix]
                ab = a_full[b][ix]
            pb_bf = pb.astype(_BF_NP).astype(np.float32)
            sg = np.where(pb_bf >= 0.5, 1.0, -1.0).astype(np.float32)
            aux[0:n, REW + off:REW + off + n] = pb
            aux[0:n, REW + FS + off:REW + FS + off + n] = ab
            aux[0:n, REW + 2 * FS + off:REW + 2 * FS + off + n] = sg
            aux[0:n, REW + 3 * FS + off:REW + 3 * FS + off + n] = 1.0
            lay.append((b, off, n, float(sg.sum())))
            off += n
        small_layout.append(lay)
        im["aux"] = aux.astype(_BF_NP)

        crd = np.zeros((128, 2 * CW), np.float32)
        v = np.zeros(128 * CW, np.float32)
        v[:len(pc_sp[c])] = pc_sp[c]
        crd[:, 0:CW] = v.reshape(128, CW)
        v = np.zeros(128 * CW, np.float32)
        v[:len(pt_sp[c])] = pt_sp[c]
        crd[:, CW:2 * CW] = v.reshape(128, CW)
        im["crd"] = crd.astype(_BF_NP)
        in_maps.append(im)

    res = run_bass_kernel_spmd(nc, in_maps, core_ids=list(range(N_CORES)))
    if _want_results is not None:
        _want_results.append(res)

    # ---- host finalization (float64) ----
    st = [res.results[c]["st"].astype(np.float64) for c in range(N_CORES)]
    dg = np.arange(128)
    pdiag = [res.results[c]["ps"].astype(np.float64) for c in range(N_CORES)]

    def diag(c, r):
        m = pdiag[c][:, 128 * r:128 * (r + 1)]
        return m[dg, dg]

    n_arr = n_list.astype(np.float64)
    cnt_coord = max(float(n_arr.sum()) * C, 1.0)
    cnt2 = max(float((n_arr ** 2).sum()), 1.0)
    LNH = float(np.log(0.5))

    s_dense = sum(float(s[:, 0:NCH].sum()) for s in st)
    s_edge = sum(float(s[:, NCH:2 * NCH].sum()) for s in st)
    npad_dense = 4.0 * DENSE_QUADS * N_CORES - len(dense_p)
    s_dense -= npad_dense * (20.0 / 19.0) * LNH
    edge_sum = 0.95 * s_dense + 0.9 * s_edge
    edge_loss = -edge_sum / cnt2

    s_r2 = sum(float(diag(c, 0).sum()) for c in range(N_CORES))
    s_re = sum(float(diag(c, 1).sum()) for c in range(N_CORES))
    similarity_loss = (s_r2 - 2.0 * s_re + e_tot) / cnt2

    s_d2 = sum(float(diag(c, 7)[0:CW].sum()) for c in range(N_CORES))
    s_h2 = sum(float(diag(c, 7)[CW:2 * CW].sum()) for c in range(N_CORES))
    coord_mse = s_d2 / cnt_coord
    coord_smooth = 0.5 * (s_d2 - s_h2) / cnt_coord
    coord_loss = 0.7 * coord_mse + 0.3 * coord_smooth

    ari_loss = 0.0
    conf_pen = 0.0
    for c in range(N_CORES):
        d2 = diag(c, 2)
        d3 = diag(c, 3)
        d4 = diag(c, 4)
        d5 = diag(c, 5)
        d6 = diag(c, 6)
        for b, off, n, sgsum in small_layout[c]:
            nf = float(n)
            sl = slice(off, off + n)
            s_p2 = float(d2[sl].sum()) - nf * (FS - nf) * 0.25
            s_pa = float(d3[sl].sum())
            s_pd = float(d4[sl].sum())
            s_abs = float(d5[sl].sum()) - 0.5 * sgsum
            s_l1p = float(d6[sl].sum())
            if not (5.0 < nf <= 50.0):
                continue
            na = np.sqrt(max(s_p2, 0.0))
            nt = np.sqrt(max(float(ec_list[b]), 0.0))
            cos = s_pa / (max(na, EPS) * max(nt, EPS))
            n2 = max(nf * nf, 1.0)
            ent = -(s_l1p + s_pd) / n2
            contrast = s_abs / n2
            ari_loss += -cos - 0.2 * contrast
            conf_pen += ent

    dc = np.asarray(node_counts, np.float64) - n_arr
    count_loss = float(_huber(dc).mean())
    temp_reg = abs(float(temperature) - 1.0)
    res_reg = abs(float(residual_weight) - 0.5)

    total = (1.0 * coord_loss + 2.0 * edge_loss + 0.1 * count_loss
             + 0.3 * similarity_loss + 0.01 * (temp_reg + res_reg)
             + 1.0 * (ari_loss + 0.1 * conf_pen))
    return np.asarray(total, dtype=np.float32)
